# revision 1
# baseline (speedup 1.0000x reference)
# Trainium2 Bass kernel for nn_AttentionLayer (GQA attention layer).
#
# Sharding: tensor-parallel over query groups. Each of the 8 cores owns one
# query group (4 Q heads + 1 KV head): it computes its slice of the QKV
# projection, rope, causal attention for its heads, and a row-parallel
# partial of the dense output projection. The 8 partial dense outputs are
# summed on the host (no collectives needed).
#
# Device layouts (T = transposed, i.e. feature-major):
#   xT      [128, 32, 4096] bf16   xT[p, hc, t] = x[t, hc*128+p]
#   wqkv    [128, 32, 768]  bf16   wqkv[p, hc, f] = Wg[hc*128+p, f]
#   wdense  [128, 4, 4096]  bf16   wd[p, c, o] = W_dense[512*g + c*128 + p, o]
#   out     [4096, 4096]    bf16   per-core partial of x @ ... @ W_dense
#
# All matmuls run in bf16 with f32 PSUM accumulation. Softmax skips the
# running-max subtraction (scores*scale have std ~1.3, max ~6; exp is safe in
# f32) so exp is purely elementwise and can be applied to transposed scores
# [k, q], which is the layout the V-matmul wants. Row sums of probs are
# accumulated on the vector engine, inverted, and broadcast back over
# partitions with a rank-1 matmul.

import numpy as np
import ml_dtypes

H = 4096
S = 2048
B = 2
TOK = B * S          # 4096 tokens
HD = 128             # head dim
NG = 8               # kv groups == cores
REP = 4              # q heads per group
T1 = 512             # phase-1 token tile
NTT = TOK // T1      # 8 token tiles
NCHUNK = 32          # 128-token chunks over TOK
SCALE = float(1.0 / np.sqrt(np.float32(HD)))

bf16 = ml_dtypes.bfloat16

_CACHE = {}


def _build_nc():
    from concourse import bacc, mybir
    import concourse.tile as tile

    dt = mybir.dt
    f32, bfd = dt.float32, dt.bfloat16
    AF = mybir.ActivationFunctionType

    nc = bacc.Bacc("TRN2", target_bir_lowering=False, debug=False, num_devices=8)

    xT_d = nc.dram_tensor("xT", [128, 32, TOK], bfd, kind="ExternalInput").ap()
    wqkv_d = nc.dram_tensor("wqkv", [128, 32, 768], bfd, kind="ExternalInput").ap()
    wd_d = nc.dram_tensor("wdense", [128, 4, H], bfd, kind="ExternalInput").ap()
    bias_d = nc.dram_tensor("biasq", [128, 6], f32, kind="ExternalInput").ap()
    cos_d = nc.dram_tensor("cosb", [128, TOK], bfd, kind="ExternalInput").ap()
    sin_d = nc.dram_tensor("ssinb", [128, TOK], bfd, kind="ExternalInput").ap()
    mask_d = nc.dram_tensor("masks", [128, 4 * 512], bfd, kind="ExternalInput").ap()
    perm_d = nc.dram_tensor("perm", [128, 128], bfd, kind="ExternalInput").ap()
    id_d = nc.dram_tensor("ident", [128, 128], bfd, kind="ExternalInput").ap()
    ob_d = nc.dram_tensor("onesb", [128, 1], bfd, kind="ExternalInput").ap()
    of_d = nc.dram_tensor("onesf", [1, 128], f32, kind="ExternalInput").ap()
    out_d = nc.dram_tensor("out", [TOK, H], bfd, kind="ExternalOutput").ap()

    with tile.TileContext(nc) as tc:
        from contextlib import ExitStack

        with ExitStack() as ctx:
            singles = ctx.enter_context(tc.tile_pool(name="singles", bufs=1))
            xt_p = ctx.enter_context(tc.tile_pool(name="xt", bufs=33))
            cs_p = ctx.enter_context(tc.tile_pool(name="cs", bufs=4))
            raw_p = ctx.enter_context(tc.tile_pool(name="raw", bufs=3))
            rt_p = ctx.enter_context(tc.tile_pool(name="rt", bufs=3))
            qt_p = ctx.enter_context(tc.tile_pool(name="qt", bufs=20))
            pb_p = ctx.enter_context(tc.tile_pool(name="pb", bufs=6))
            sp_p = ctx.enter_context(tc.tile_pool(name="sp", bufs=2))
            sb_p = ctx.enter_context(tc.tile_pool(name="sbf", bufs=2))
            rc_p = ctx.enter_context(tc.tile_pool(name="rc", bufs=2))
            rb_p = ctx.enter_context(tc.tile_pool(name="rb", bufs=2))
            nm_p = ctx.enter_context(tc.tile_pool(name="nm", bufs=6))
            os_p = ctx.enter_context(tc.tile_pool(name="os", bufs=2))
            ps_mm = ctx.enter_context(tc.tile_pool(name="ps_mm", bufs=2, space="PSUM"))
            ps_att = ctx.enter_context(tc.tile_pool(name="ps_att", bufs=2, space="PSUM"))
            ps_prm = ctx.enter_context(tc.tile_pool(name="ps_prm", bufs=1, space="PSUM"))
            ps_vt = ctx.enter_context(tc.tile_pool(name="ps_vt", bufs=1, space="PSUM"))
            ps_o = ctx.enter_context(tc.tile_pool(name="ps_o", bufs=1, space="PSUM"))
            ps_sb = ctx.enter_context(tc.tile_pool(name="ps_sb", bufs=1, space="PSUM"))

            # resident constants and K/V
            wqkv = singles.tile([128, 32, 768], bfd, name="wqkv_sb")
            nc.sync.dma_start(out=wqkv, in_=wqkv_d)
            wd = singles.tile([128, 4, H], bfd, name="wd_sb")
            nc.sync.dma_start(out=wd, in_=wd_d)
            bias = singles.tile([128, 6], f32, name="bias_sb")
            nc.sync.dma_start(out=bias, in_=bias_d)
            masks = singles.tile([128, 4 * 512], bfd, name="masks_sb")
            nc.sync.dma_start(out=masks, in_=mask_d)
            perm = singles.tile([128, 128], bfd, name="perm_sb")
            nc.sync.dma_start(out=perm, in_=perm_d)
            ident = singles.tile([128, 128], bfd, name="ident_sb")
            nc.sync.dma_start(out=ident, in_=id_d)
            onesb = singles.tile([128, 1], bfd, name="onesb_sb")
            nc.sync.dma_start(out=onesb, in_=ob_d)
            onesf = singles.tile([1, 128], f32, name="onesf_sb")
            nc.sync.dma_start(out=onesf, in_=of_d)
            KT = singles.tile([128, TOK], bfd, name="KT_sb")
            VC = singles.tile([128, TOK], bfd, name="VC_sb")

            qtiles = {}

            def qkv_tok_tile(tt):
                sl = slice(tt * T1, (tt + 1) * T1)
                xts = []
                for hc in range(32):
                    t = xt_p.tile([128, T1], bfd, name=f"xt_{tt}_{hc}", tag="xt")
                    nc.sync.dma_start(out=t, in_=xT_d[:, hc, sl])
                    xts.append(t)
                cost = cs_p.tile([128, T1], bfd, name=f"cos_{tt}", tag="cs")
                nc.sync.dma_start(out=cost, in_=cos_d[:, sl])
                sint = cs_p.tile([128, T1], bfd, name=f"sin_{tt}", tag="cs")
                nc.sync.dma_start(out=sint, in_=sin_d[:, sl])
                for ft in range(6):
                    ps = ps_mm.tile([128, T1], f32, name=f"psq_{tt}_{ft}", tag="ps_mm")
                    for hc in range(32):
                        nc.tensor.matmul(
                            ps,
                            lhsT=wqkv[:, hc, ft * 128 : (ft + 1) * 128],
                            rhs=xts[hc],
                            start=(hc == 0),
                            stop=(hc == 31),
                        )
                    if ft < 5:
                        raw = raw_p.tile([128, T1], bfd, name=f"raw_{tt}_{ft}", tag="raw")
                        nc.scalar.activation(
                            raw, ps, AF.Identity, bias=bias[:, ft : ft + 1], scale=1.0
                        )
                        pp = ps_prm.tile([128, T1], f32, name=f"pp_{tt}_{ft}", tag="ps_prm")
                        nc.tensor.matmul(pp, lhsT=perm, rhs=raw, start=True, stop=True)
                        t1 = rt_p.tile([128, T1], f32, name=f"t1_{tt}_{ft}", tag="rt")
                        nc.vector.tensor_mul(t1, raw, cost)
                        t2 = rt_p.tile([128, T1], f32, name=f"t2_{tt}_{ft}", tag="rt")
                        nc.vector.tensor_mul(t2, pp, sint)
                        if ft < 4:
                            qt_t = qt_p.tile([128, T1], bfd, name=f"q_{tt}_{ft}", tag="qt")
                            nc.vector.tensor_add(qt_t, t1, t2)
                            qtiles[(ft, tt)] = qt_t
                        else:
                            nc.vector.tensor_add(KT[:, sl], t1, t2)
                    else:
                        vraw = raw_p.tile([128, T1], bfd, name=f"vraw_{tt}", tag="raw")
                        nc.scalar.activation(
                            vraw, ps, AF.Identity, bias=bias[:, 5:6], scale=1.0
                        )
                        for j in range(4):
                            pv = ps_vt.tile([128, 128], bfd, name=f"pv_{tt}_{j}", tag="ps_vt")
                            nc.tensor.transpose(pv, vraw[:, j * 128 : (j + 1) * 128], ident)
                            ch = tt * 4 + j
                            nc.vector.tensor_copy(VC[:, ch * 128 : (ch + 1) * 128], pv)

            def attn_group(b, h, qt):
                # queries [qt*512, (qt+1)*512) of batch b, head h; keys 0..(qt+1)*512
                nkt = 4 * qt + 4
                qt_t = qtiles[(h, b * 4 + qt)]
                S_part = sp_p.tile([128, 512], f32, name=f"spart_{b}_{h}_{qt}", tag="sp")
                po = ps_o.tile([128, 512], f32, name=f"po_{b}_{h}_{qt}", tag="ps_o")
                prev = None
                for kt in range(nkt):
                    ch = b * 16 + kt
                    pss = ps_att.tile(
                        [128, 512], f32, name=f"pss_{b}_{h}_{qt}_{kt}", tag="ps_att"
                    )
                    nc.tensor.matmul(
                        pss,
                        lhsT=KT[:, ch * 128 : (ch + 1) * 128],
                        rhs=qt_t,
                        start=True,
                        stop=True,
                    )
                    pb = pb_p.tile([128, 512], bfd, name=f"pb_{b}_{h}_{qt}_{kt}", tag="pb")
                    nc.scalar.activation(pb, pss, AF.Exp, scale=SCALE)
                    r = kt - 4 * qt
                    if r >= 0:
                        nc.vector.tensor_mul(pb, pb, masks[:, r * 512 : (r + 1) * 512])
                    if kt == 0:
                        nc.vector.tensor_copy(S_part, pb)
                    else:
                        nc.vector.tensor_add(S_part, S_part, pb)
                    if prev is not None:
                        kp, pbp = prev
                        nc.tensor.matmul(
                            po,
                            lhsT=VC[:, (b * 16 + kp) * 128 : (b * 16 + kp + 1) * 128],
                            rhs=pbp,
                            start=(kp == 0),
                            stop=False,
                        )
                    prev = (kt, pb)
                kp, pbp = prev
                nc.tensor.matmul(
                    po,
                    lhsT=VC[:, (b * 16 + kp) * 128 : (b * 16 + kp + 1) * 128],
                    rhs=pbp,
                    start=(kp == 0),
                    stop=True,
                )
                S_bf = sb_p.tile([128, 512], bfd, name=f"sbf_{b}_{h}_{qt}", tag="sbf")
                nc.vector.tensor_copy(S_bf, S_part)
                ps_sum = ps_sb.tile([1, 512], f32, name=f"psum_{b}_{h}_{qt}", tag="ps_sb")
                nc.tensor.matmul(ps_sum, lhsT=onesb, rhs=S_bf, start=True, stop=True)
                rec = rc_p.tile([1, 512], f32, name=f"rec_{b}_{h}_{qt}", tag="rc")
                nc.vector.reciprocal(rec, ps_sum)
                ps_bc = ps_sb.tile([128, 512], f32, name=f"pbc_{b}_{h}_{qt}", tag="ps_sb")
                nc.tensor.matmul(ps_bc, lhsT=onesf, rhs=rec, start=True, stop=True)
                rb = rb_p.tile([128, 512], f32, name=f"rb_{b}_{h}_{qt}", tag="rb")
                nc.scalar.copy(rb, ps_bc)
                nm = nm_p.tile([128, 512], bfd, name=f"nm_{b}_{h}_{qt}", tag="nm")
                nc.vector.tensor_mul(nm, po, rb)
                return nm

            def dense_chunk(b, qt, nms):
                for tl in range(4):
                    row = b * 2048 + qt * 512 + tl * 128
                    for half in range(2):
                        ost = os_p.tile(
                            [128, 2048], bfd, name=f"ost_{b}_{qt}_{tl}_{half}", tag="os"
                        )
                        for oi in range(4):
                            ot = half * 4 + oi
                            pd = ps_mm.tile(
                                [128, 512], f32, name=f"pd_{b}_{qt}_{tl}_{ot}", tag="ps_mm"
                            )
                            for c in range(4):
                                nc.tensor.matmul(
                                    pd,
                                    lhsT=nms[c][:, tl * 128 : (tl + 1) * 128],
                                    rhs=wd[:, c, ot * 512 : (ot + 1) * 512],
                                    start=(c == 0),
                                    stop=(c == 3),
                                )
                            nc.scalar.copy(ost[:, oi * 512 : (oi + 1) * 512], pd)
                        nc.sync.dma_start(
                            out=out_d[row : row + 128, half * 2048 : (half + 1) * 2048],
                            in_=ost,
                        )

            for b in range(B):
                for tt in range(b * 4, b * 4 + 4):
                    qkv_tok_tile(tt)
                for qt in range(4):
                    nms = [attn_group(b, h, qt) for h in range(4)]
                    dense_chunk(b, qt, nms)

    nc.compile()
    return nc


def _host_prep(x, rope_cache, W_qkv, b_qkv, W_dense):
    x = np.asarray(x, np.float32).reshape(TOK, H)
    rc = np.asarray(rope_cache, np.float32)
    Wq = np.asarray(W_qkv, np.float32)
    bq = np.asarray(b_qkv, np.float32)
    Wd = np.asarray(W_dense, np.float32)

    # xT[p, hc, t] = x[t, hc*128 + p]
    xT = np.ascontiguousarray(
        x.T.reshape(32, 128, TOK).transpose(1, 0, 2)
    ).astype(bf16)

    # rope tables: cos/ssin [128, TOK]
    c = rc[:, :, 0].T  # [64, S]
    s = rc[:, :, 1].T
    cos = np.empty((HD, TOK), np.float32)
    ssin = np.empty((HD, TOK), np.float32)
    for b in range(B):
        sl = slice(b * S, (b + 1) * S)
        cos[0::2, sl] = c
        cos[1::2, sl] = c
        ssin[0::2, sl] = -s
        ssin[1::2, sl] = s
    cosb = cos.astype(bf16)
    ssinb = ssin.astype(bf16)

    # diagonal causal mask tiles: masks[ki, r*512 + qi] = (ki + r*128 <= qi)
    ki = np.arange(128)[:, None]
    qi = np.arange(512)[None, :]
    masks = np.concatenate(
        [(ki + r * 128 <= qi).astype(np.float32) for r in range(4)], axis=1
    ).astype(bf16)

    perm = np.zeros((128, 128), np.float32)
    perm[np.arange(128), np.arange(128) ^ 1] = 1.0
    perm = perm.astype(bf16)
    ident = np.eye(128, dtype=np.float32).astype(bf16)
    onesb = np.ones((128, 1), np.float32).astype(bf16)
    onesf = np.ones((1, 128), np.float32)

    shared = {
        "xT": xT,
        "cosb": cosb,
        "ssinb": ssinb,
        "masks": masks,
        "perm": perm,
        "ident": ident,
        "onesb": onesb,
        "onesf": onesf,
    }

    in_maps = []
    for g in range(NG):
        qcols = np.arange(g * 512, (g + 1) * 512)
        kcols = np.arange(H + g * HD, H + (g + 1) * HD)
        vcols = np.arange(H + NG * HD + g * HD, H + NG * HD + (g + 1) * HD)
        cols = np.concatenate([qcols, kcols, vcols])
        Wg = Wq[:, cols]  # [H, 768]
        bg = bq[cols]
        wqkv = np.ascontiguousarray(
            Wg.reshape(32, 128, 768).transpose(1, 0, 2)
        ).astype(bf16)
        biasq = np.ascontiguousarray(bg.reshape(6, 128).T).astype(np.float32)
        wdense = np.ascontiguousarray(
            Wd[g * 512 : (g + 1) * 512].reshape(4, 128, H).transpose(1, 0, 2)
        ).astype(bf16)
        m = dict(shared)
        m.update({"wqkv": wqkv, "biasq": biasq, "wdense": wdense})
        in_maps.append(m)
    return in_maps


def kernel(x, rope_cache, W_qkv, b_qkv, W_dense):
    from concourse.bass_utils import run_bass_kernel_spmd

    if "nc" not in _CACHE:
        _CACHE["nc"] = _build_nc()
    nc = _CACHE["nc"]

    in_maps = _host_prep(x, rope_cache, W_qkv, b_qkv, W_dense)
    res = run_bass_kernel_spmd(nc, in_maps, core_ids=list(range(NG)))
    out = np.zeros((TOK, H), np.float32)
    for r in res.results:
        out += np.asarray(r["out"]).astype(np.float32)
    return out.reshape(B, S, H)
